# revision 13
# baseline (speedup 1.0000x reference)
"""AIG triple embedding layer on 8 TRN2 NeuronCores.

Math: out[t] = W @ concat(src[t], r[t], dst[t]) + b
            = W1 @ table[fs[t]] + (W2 @ edge[rel[t]] + b) + W3 @ table[fd[t]]
where table = [inp_enc(256) | out_enc(256) | gate[:256] | zeros], and
fs = src_type*256 + src_idx (type==3 rows land in the zero pad).

Because indices are bounded (idx < 256, 4 types, rel in {0,1}) the linear
layer is folded into two small tables computed on device:
  TA[r + v*1024] = table1024[r] @ W1.T + edge[v] @ W2.T + b   (2048 rows)
  TB[r]          = table1024[r] @ W3.T                        (1024 rows)
then  out[t] = TA[fs[t] + rel[t]*1024] + TB[fd[t]]  -- two row gathers + add.

Sharding: data-parallel over T across 8 cores; tables/weights replicated.
"""

import numpy as np

D = 128
T = 524288
NCORES = 8
NSHARD = T // NCORES  # 65536
NI = 256              # num_input_nodes == num_output_nodes == IDX_MAX
ROWS = 4 * NI         # 1024 padded flat-table rows (type*256 + idx < 1024)
P = 128
BLK = 8192            # triples per pipeline block
NBLK = NSHARD // BLK  # 8
JB = BLK // P         # 64 rows per partition per block
FB = BLK // 16        # 512 idx columns per block in the [16, *] wrapped layout
G = 1024              # indices per dma_gather call (SWDGE ring holds <2048)
NG = BLK // G         # 8 gather sub-calls per block per table
NF = NSHARD // 16     # 4096 idx columns whole-shard

USE_BF16 = True

_CACHE = {}


def _sinusoid(n, d):
    pos = np.arange(n, dtype=np.float32)[:, None]
    div = np.exp(np.arange(0, d, 2, dtype=np.float32)
                 * (-np.log(np.float32(10000.0)) / np.float32(d)))
    ang = (pos * div).astype(np.float32)
    enc = np.zeros((n, d), np.float32)
    enc[:, 0::2] = np.sin(ang)
    enc[:, 1::2] = np.cos(ang)
    return enc


def _build_nc():
    import concourse.bacc as bacc
    import concourse.mybir as mybir
    import concourse.tile as tile

    f32 = mybir.dt.float32
    tdt = mybir.dt.bfloat16 if USE_BF16 else mybir.dt.float32
    i32 = mybir.dt.int32
    i16 = mybir.dt.int16
    AL = mybir.AluOpType

    nc = bacc.Bacc(None, target_bir_lowering=False)

    tblT = nc.dram_tensor("tblT", [P, ROWS], f32, kind="ExternalInput")
    wt = nc.dram_tensor("wt", [3 * D, D], f32, kind="ExternalInput")
    edgt = nc.dram_tensor("edget", [P, 2], f32, kind="ExternalInput")
    bv = nc.dram_tensor("bvec", [1, D], f32, kind="ExternalInput")
    s_i = nc.dram_tensor("src_idx", [NSHARD], i32, kind="ExternalInput")
    s_t = nc.dram_tensor("src_type", [NSHARD], i32, kind="ExternalInput")
    r_l = nc.dram_tensor("rel", [NSHARD], i32, kind="ExternalInput")
    d_i = nc.dram_tensor("dst_idx", [NSHARD], i32, kind="ExternalInput")
    d_t = nc.dram_tensor("dst_type", [NSHARD], i32, kind="ExternalInput")
    out = nc.dram_tensor("out", [NSHARD, D], f32, kind="ExternalOutput")

    # Triple t sits at gathered position (block b, call k, q) with
    # p = q%128 = 16*w2 + pi, j = k*8 + q//128 = k*8 + u2, and the output
    # write keeps partition p's 64 rows contiguous: t = p*512 + b*64 + j.
    # With m = b*64 + k*8 + u2 in [0,512):  t = w2*8192 + pi*512 + m.
    # Whole-shard idx arrays load once into [16, 4096] SBUF as f' = w2*512+m
    # (both sides clean 3-dim APs, 2KB contiguous DRAM runs); the int16 cast
    # later permutes to the wrapped per-call order f = m*8 + w2.
    def idx_view(h):
        return h[:].rearrange("(w2 pi m) -> pi w2 m", w2=8, pi=16, m=512)

    outv = out[:].rearrange("(p b j) d -> b p (j d)", p=P, b=NBLK, j=JB)

    with tile.TileContext(nc) as tc:
        with (
            tc.tile_pool(name="const", bufs=1) as cpool,
            tc.tile_pool(name="psum", bufs=2, space="PSUM") as psum,
            tc.tile_pool(name="setup", bufs=2) as spool,
            tc.tile_pool(name="idxin", bufs=2) as iip,
            tc.tile_pool(name="idxmath", bufs=2) as imp,
            tc.tile_pool(name="idxrep", bufs=2) as irp,
            tc.tile_pool(name="gather", bufs=2) as gpool,
            tc.tile_pool(name="sum", bufs=2) as opool,
            tc.tile_pool(name="dram", bufs=1, space="DRAM") as dpool,
        ):
            # ---------------- fused tables (one-time, tiny) ----------------
            TA = dpool.tile([2 * ROWS, D], tdt)
            TB = dpool.tile([ROWS, D], tdt)

            tblT_sb = cpool.tile([P, ROWS], f32)
            nc.sync.dma_start(out=tblT_sb[:], in_=tblT[:])
            wt_sb = cpool.tile([P, 3 * D], f32)  # three [128,128] chunks
            for k in range(3):
                nc.sync.dma_start(out=wt_sb[:, k * D:(k + 1) * D],
                                  in_=wt[k * D:(k + 1) * D, :])
            edgt_sb = cpool.tile([P, 2], f32)
            nc.sync.dma_start(out=edgt_sb[:], in_=edgt[:])
            b_sb = cpool.tile([1, D], f32)
            nc.sync.dma_start(out=b_sb[:], in_=bv[:])
            ones2 = cpool.tile([1, 2], f32)
            nc.gpsimd.memset(ones2[:], 1.0)
            onesM = cpool.tile([1, P], f32)
            nc.gpsimd.memset(onesM[:], 1.0)

            # EW2'[v] = edge[v] @ W2.T + b   (one [1,D] row per v, so each
            # lands at base partition 0 as required for matmul rhs use)
            ew = []
            for v in range(2):
                pe = psum.tile([1, D], f32, tag="pe")
                nc.tensor.matmul(out=pe[:], lhsT=edgt_sb[:, v:v + 1],
                                 rhs=wt_sb[:, D:2 * D], start=True, stop=False)
                nc.tensor.matmul(out=pe[:], lhsT=ones2[:, 0:1], rhs=b_sb[:],
                                 start=False, stop=True)
                ewv = cpool.tile([1, D], f32, tag=f"ew{v}")
                nc.vector.tensor_copy(out=ewv[:], in_=pe[:])
                ew.append(ewv)

            for c in range(ROWS // P):  # 8 chunks of 128 table rows
                lhs = tblT_sb[:, c * P:(c + 1) * P]
                for v in range(2):  # TA halves: + EW2'[v]
                    pa = psum.tile([P, D], f32, tag="pa")
                    nc.tensor.matmul(out=pa[:], lhsT=lhs,
                                     rhs=wt_sb[:, 0:D], start=True, stop=False)
                    nc.tensor.matmul(out=pa[:], lhsT=onesM[:],
                                     rhs=ew[v][:], start=False, stop=True)
                    av = spool.tile([P, D], tdt, tag="av")
                    nc.vector.tensor_copy(out=av[:], in_=pa[:])
                    nc.sync.dma_start(
                        out=TA[v * ROWS + c * P: v * ROWS + (c + 1) * P, :],
                        in_=av[:])
                pb = psum.tile([P, D], f32, tag="pa")
                nc.tensor.matmul(out=pb[:], lhsT=lhs,
                                 rhs=wt_sb[:, 2 * D:3 * D], start=True, stop=True)
                bt = spool.tile([P, D], tdt, tag="av")
                nc.vector.tensor_copy(out=bt[:], in_=pb[:])
                nc.sync.dma_start(out=TB[c * P:(c + 1) * P, :], in_=bt[:])

            # ---------------- per-block pipeline ----------------
            for bb in range(NBLK):
                sti = iip.tile([16, FB], i32, tag="sti")
                sii = iip.tile([16, FB], i32, tag="sii")
                rli = iip.tile([16, FB], i32, tag="rli")
                dti = iip.tile([16, FB], i32, tag="dti")
                dii = iip.tile([16, FB], i32, tag="dii")

                def split(t):  # [16, FB] -> [16, 8, 64] contiguous view
                    return t[:].rearrange("pi (w2 mw) -> pi w2 mw", w2=8, mw=JB)

                for tl, h in ((sti, s_t), (sii, s_i), (rli, r_l),
                              (dti, d_t), (dii, d_i)):
                    nc.sync.dma_start(
                        out=split(tl),
                        in_=idx_view(h)[:, :, bb * JB:(bb + 1) * JB])

                # fs = st*256 + si + rel*1024 ; fd = dt*256 + di
                fs32 = imp.tile([16, FB], i32, tag="fs32")
                t32 = imp.tile([16, FB], i32, tag="t32")
                fd32 = imp.tile([16, FB], i32, tag="fd32")
                nc.vector.tensor_scalar(out=fs32[:], in0=sti[:], scalar1=8,
                                        scalar2=None, op0=AL.logical_shift_left)
                nc.vector.tensor_tensor(out=fs32[:], in0=fs32[:], in1=sii[:],
                                        op=AL.add)
                nc.vector.tensor_scalar(out=t32[:], in0=rli[:], scalar1=10,
                                        scalar2=None, op0=AL.logical_shift_left)
                nc.vector.tensor_tensor(out=fs32[:], in0=fs32[:], in1=t32[:],
                                        op=AL.add)
                nc.vector.tensor_scalar(out=fd32[:], in0=dti[:], scalar1=8,
                                        scalar2=None, op0=AL.logical_shift_left)
                nc.vector.tensor_tensor(out=fd32[:], in0=fd32[:], in1=dii[:],
                                        op=AL.add)

                # cast to int16, permuting storage f''=w2*64+mw into the
                # wrapped per-call order f = mw*8 + w2
                def pmw(t):  # [16, 64, 8] permuted view of f'' = w2*64 + mw
                    return t[:].rearrange("pi (w2 mw) -> pi mw w2",
                                          w2=8, mw=JB)

                fsd16 = imp.tile([16, 2 * FB], i16, tag="fsd16")
                nc.vector.tensor_copy(
                    out=fsd16[:, 0:FB].rearrange("pi (mw w2) -> pi mw w2",
                                                 mw=JB, w2=8),
                    in_=pmw(fs32))
                nc.vector.tensor_copy(
                    out=fsd16[:, FB:2 * FB].rearrange("pi (mw w2) -> pi mw w2",
                                                      mw=JB, w2=8),
                    in_=pmw(fd32))

                # replicate across the 8 Q7-core partition groups
                rep = irp.tile([P, 2 * FB], i16, tag="rep")
                for g in range(8):
                    nc.sync.dma_start(out=rep[g * 16:(g + 1) * 16, :],
                                      in_=fsd16[:])

                ga = gpool.tile([P, JB, D], tdt, tag="ga")
                gb = gpool.tile([P, JB, D], tdt, tag="gb")
                for k in range(NG):
                    nc.gpsimd.dma_gather(
                        ga[:, k * (G // P):(k + 1) * (G // P), :], TA[:],
                        rep[:, k * (G // 16):(k + 1) * (G // 16)], G, G, D)
                    nc.gpsimd.dma_gather(
                        gb[:, k * (G // P):(k + 1) * (G // P), :], TB[:],
                        rep[:, FB + k * (G // 16):FB + (k + 1) * (G // 16)],
                        G, G, D)

                s = opool.tile([P, JB * D], f32, tag="s")
                nc.vector.tensor_tensor(
                    out=s[:],
                    in0=ga[:].rearrange("p a b -> p (a b)"),
                    in1=gb[:].rearrange("p a b -> p (a b)"),
                    op=AL.add)
                nc.sync.dma_start(out=outv[bb], in_=s[:])

    nc.compile()
    return nc


def _make_in_maps(inputs):
    gate = np.asarray(inputs["gate_emb"], np.float32)
    edge = np.asarray(inputs["edge_emb"], np.float32)
    W = np.asarray(inputs["W"], np.float32)
    b = np.asarray(inputs["b"], np.float32)

    tbl = np.concatenate([
        _sinusoid(NI, D),
        _sinusoid(NI, D),
        gate[:NI],
        np.zeros((ROWS - 3 * NI, D), np.float32),
    ], axis=0)  # [1024, 128]

    common = {
        "tblT": np.ascontiguousarray(tbl.T),
        "wt": np.ascontiguousarray(W.T),
        "edget": np.ascontiguousarray(edge.T),
        "bvec": np.ascontiguousarray(b.reshape(1, D)),
    }
    idx_names = ["src_idx", "src_type", "rel", "dst_idx", "dst_type"]
    idx = {k: np.ascontiguousarray(np.asarray(inputs[k]).astype(np.int32))
           for k in idx_names}

    in_maps = []
    for c in range(NCORES):
        m = dict(common)
        for k in idx_names:
            m[k] = np.ascontiguousarray(idx[k][c * NSHARD:(c + 1) * NSHARD])
        in_maps.append(m)
    return in_maps


def kernel(**inputs):
    from concourse.bass_utils import run_bass_kernel_spmd

    if "nc" not in _CACHE:
        _CACHE["nc"] = _build_nc()
    nc = _CACHE["nc"]

    in_maps = _make_in_maps(inputs)
    res = run_bass_kernel_spmd(nc, in_maps, core_ids=list(range(NCORES)))
    return np.concatenate([res.results[c]["out"] for c in range(NCORES)],
                          axis=0)
